# revision 4
# baseline (speedup 1.0000x reference)
"""Trainium2 Bass kernel for nn_ConcatAttentionLayer.

Reference computation (S=2048, B=64, D_in=D_src=D_align=512):
    cat    = concat(broadcast(input), source_hids)        # [S, B, 1024]
    h      = tanh(cat @ W1.T + b1)                        # [S, B, 512]
    scores = h @ W2.T + b2                                # [S, B]
    attn   = softmax(scores, axis=0)                      # [S, B]  (b2 cancels)
    ctx    = einsum('sb,sbd->bd', attn, source_hids)      # [B, 512]
    returns (ctx, attn)

Sharding: data-parallel over the batch axis, 8 batch columns per core on
8 NeuronCores; the softmax/weighted-sum reduction over srclen stays local.

Device mapping (per core, bs=8 local batch):
  mm1  HT[a,s]    = W1s @ src[s,b,:].T      fp32r matmuls, PE
  tanh th = tanh(HT + (W1x@input.T + b1))   ScalarE, bias folded per-partition
  mm2  scores[s]  = W2 . th                 PE, M=1
  softmax over s (free axis)                DVE + ScalarE exp
  mm3  ctx[b,:]   = sum_s attn[s,b]*src     PE, src in natural (s-major) layout

The host passes source_hids twice in two layouts (pure relayout, no compute):
d-major ("xt") for mm1 and s-major ("xn") for mm3.
"""

import numpy as np

S, B, D = 2048, 64, 512
NCORES = 8
BS = B // NCORES          # 8 batch columns per core
ND = NA = D // 128        # 4 contraction / output chunks
NT = S // 128             # 16 s-chunks (phase C)
ST = 512                  # s-tile width (phase A)
NST = S // ST             # 4

_NC_CACHE = {}


def _build_nc():
    import concourse.bass as bass
    import concourse.bacc as bacc
    import concourse.tile as tile
    from concourse import mybir
    from concourse.masks import make_identity
    from contextlib import ExitStack

    f32 = mybir.dt.float32
    f32r = mybir.dt.float32r
    AF = mybir.ActivationFunctionType
    AX = mybir.AxisListType
    ts = bass.ts

    nc = bacc.Bacc(
        "TRN2",
        target_bir_lowering=False,
        debug=False,
        enable_asserts=False,
        num_devices=NCORES,
    )

    xt_d = nc.declare_dram_parameter("xt", [BS, 128, ND, S], f32r, False)
    xn_d = nc.declare_dram_parameter("xn", [BS, 128, NT, D], f32r, False)
    w1st_d = nc.declare_dram_parameter("w1st", [128, ND, D], f32r, False)
    w1xt_d = nc.declare_dram_parameter("w1xt", [128, ND, D], f32, False)
    w2t_d = nc.declare_dram_parameter("w2t", [128, NA], f32r, False)
    b1t_d = nc.declare_dram_parameter("b1t", [128, NA], f32, False)
    inpT_d = nc.declare_dram_parameter("inpT", [128, ND, BS], f32, False)
    ctx_o = nc.declare_dram_parameter("ctx_o", [BS, D], f32, True)
    attn_o = nc.declare_dram_parameter("attn_o", [S, BS], f32, True)

    with tile.TileContext(nc) as tc, ExitStack() as ectx:
        consts = ectx.enter_context(tc.tile_pool(name="consts", bufs=1))
        xtp = ectx.enter_context(tc.tile_pool(name="xtp", bufs=3))
        xnp = ectx.enter_context(tc.tile_pool(name="xnp", bufs=3))
        thp = ectx.enter_context(tc.tile_pool(name="thp", bufs=5))
        stage = ectx.enter_context(tc.tile_pool(name="stage", bufs=4))
        psum = ectx.enter_context(tc.tile_pool(name="psum", bufs=1, space="PSUM"))

        w1st = consts.tile([128, ND, D], f32r)
        nc.sync.dma_start(w1st[:], w1st_d[:])
        w1xt = consts.tile([128, ND, D], f32)
        nc.sync.dma_start(w1xt[:], w1xt_d[:])
        w2t = consts.tile([128, NA], f32r)
        nc.sync.dma_start(w2t[:], w2t_d[:])
        b1t = consts.tile([128, NA], f32)
        nc.sync.dma_start(b1t[:], b1t_d[:])
        inpT = consts.tile([128, ND, BS], f32)
        nc.sync.dma_start(inpT[:], inpT_d[:])
        ident = consts.tile([128, 128], f32)
        make_identity(nc, ident[:])

        hxb1 = consts.tile([128, NA, BS], f32)
        scores = consts.tile([BS, S], f32)
        attn = consts.tile([BS, S], f32)
        attnT = consts.tile([128, NT, BS], f32r)
        attnTf = consts.tile([128, NT, BS], f32)
        mx = consts.tile([BS, 1], f32)
        nmx = consts.tile([BS, 1], f32)
        sm = consts.tile([BS, 1], f32)
        rs = consts.tile([BS, 1], f32)

        # mm0: hxb1[p, ac, n] = sum_d W1x[a, d] * input[b0+n, d] + b1[a],  a = ac*128+p
        for ac in range(NA):
            hx_ps = psum.tile([128, BS], f32, tag="ht", bufs=4)
            for dc in range(ND):
                nc.tensor.matmul(
                    hx_ps[:],
                    w1xt[:, dc, ts(ac, 128)],
                    inpT[:, dc, :],
                    start=(dc == 0),
                    stop=(dc == ND - 1),
                )
            nc.vector.tensor_scalar_add(hxb1[:, ac, :], hx_ps[:], b1t[:, ac : ac + 1])

        # phase A: scores
        for b in range(BS):
            for st in range(NST):
                xt_t = xtp.tile([128, ND, ST], f32r)
                nc.sync.dma_start(xt_t[:], xt_d[b][:, :, ts(st, ST)])
                ths = []
                for ac in range(NA):
                    ht = psum.tile([128, ST], f32, tag="ht", bufs=4)
                    for dc in range(ND):
                        nc.tensor.matmul(
                            ht[:],
                            w1st[:, dc, ts(ac, 128)],
                            xt_t[:, dc, :],
                            start=(dc == 0),
                            stop=(dc == ND - 1),
                        )
                    th = thp.tile([128, ST], f32r, tag="th")
                    nc.scalar.activation(th[:], ht[:], AF.Tanh, bias=hxb1[:, ac, b : b + 1])
                    ths.append(th)
                sc = psum.tile([1, ST], f32, tag="sc", bufs=2)
                for ac in range(NA):
                    nc.tensor.matmul(
                        sc[:],
                        w2t[:, ac : ac + 1],
                        ths[ac][:],
                        start=(ac == 0),
                        stop=(ac == NA - 1),
                    )
                stmp = stage.tile([1, ST], f32, tag="stmp")
                nc.vector.tensor_copy(stmp[:], sc[:])
                nc.sync.dma_start(scores[b : b + 1, ts(st, ST)], stmp[:])

        # phase B: softmax over s (free axis) for all 8 local batch rows at once
        nc.vector.reduce_max(mx[:], scores[:], axis=AX.X)
        nc.vector.tensor_scalar_mul(nmx[:], mx[:], -1.0)
        nc.scalar.activation(attn[:], scores[:], AF.Exp, bias=nmx[:])
        nc.vector.reduce_sum(sm[:], attn[:], axis=AX.X)
        nc.vector.reciprocal(rs[:], sm[:])
        nc.vector.tensor_scalar_mul(attn[:], attn[:], rs[:])
        for t in range(NT):
            tp = psum.tile([128, BS], f32, tag="sc", bufs=2)
            nc.tensor.transpose(tp[:], attn[:, ts(t, 128)], ident[:BS, :BS])
            nc.vector.tensor_copy(attnT[:, t, :], tp[:])
            nc.vector.tensor_copy(attnTf[:, t, :], tp[:])
        nc.sync.dma_start(attn_o[:].rearrange("(t p) b -> p t b", p=128), attnTf[:])

        # phase C: ctx[b, :] = sum_s attn[s, b] * src[s, b, :]
        for b in range(BS):
            xn_t = xnp.tile([128, NT, D], f32r)
            nc.sync.dma_start(xn_t[:], xn_d[b])
            cx = psum.tile([1, D], f32, tag="cx", bufs=2)
            for t in range(NT):
                nc.tensor.matmul(
                    cx[:],
                    attnT[:, t, b : b + 1],
                    xn_t[:, t, :],
                    start=(t == 0),
                    stop=(t == NT - 1),
                )
            ctmp = stage.tile([1, D], f32, tag="ctmp")
            nc.vector.tensor_copy(ctmp[:], cx[:])
            nc.sync.dma_start(ctx_o[b : b + 1, :], ctmp[:])

    nc.compile()
    return nc


def _get_nc():
    if "nc" not in _NC_CACHE:
        _NC_CACHE["nc"] = _build_nc()
    return _NC_CACHE["nc"]


def _prep_in_maps(input, source_hids, W1, b1, W2):
    src = np.ascontiguousarray(np.asarray(source_hids, dtype=np.float32))
    inp = np.ascontiguousarray(np.asarray(input, dtype=np.float32)[0])  # [64, 512]
    W1 = np.asarray(W1, dtype=np.float32)
    b1 = np.asarray(b1, dtype=np.float32)
    W2 = np.asarray(W2, dtype=np.float32)

    W1x, W1s = W1[:, :D], W1[:, D:]
    w1xt = np.ascontiguousarray(W1x.T.reshape(ND, 128, D).transpose(1, 0, 2))
    w1st = np.ascontiguousarray(W1s.T.reshape(ND, 128, D).transpose(1, 0, 2))
    w2t = np.ascontiguousarray(W2[0].reshape(NA, 128).T)
    b1t = np.ascontiguousarray(b1.reshape(NA, 128).T)

    in_maps = []
    for c in range(NCORES):
        b0 = c * BS
        sc_ = src[:, b0 : b0 + BS, :]  # [2048, 8, 512]
        xt = np.ascontiguousarray(
            sc_.transpose(1, 2, 0).reshape(BS, ND, 128, S).transpose(0, 2, 1, 3)
        )  # [8, 128, 4, 2048]: xt[j, p, dc, s] = src[s, b0+j, dc*128+p]
        xn = np.ascontiguousarray(
            sc_.transpose(1, 0, 2).reshape(BS, NT, 128, D).transpose(0, 2, 1, 3)
        )  # [8, 128, 16, 512]: xn[j, p, t, d] = src[t*128+p, b0+j, d]
        inpT = np.ascontiguousarray(
            inp[b0 : b0 + BS].T.reshape(ND, 128, BS).transpose(1, 0, 2)
        )  # [128, 4, 8]
        in_maps.append(
            {
                "xt": xt,
                "xn": xn,
                "w1st": w1st,
                "w1xt": w1xt,
                "w2t": w2t,
                "b1t": b1t,
                "inpT": inpT,
            }
        )
    return in_maps


def _run(in_maps, trace=False, **kwargs):
    from concourse.bass_utils import run_bass_kernel_spmd

    nc = _get_nc()
    return run_bass_kernel_spmd(nc, in_maps, list(range(NCORES)), trace=trace, **kwargs)


def kernel(input, source_hids, encoder_padding_mask, W1, b1, W2, b2):
    # encoder_padding_mask is all-False for this problem spec (fill: zeros) and
    # b2 is a constant shift that cancels in the softmax; neither affects output.
    in_maps = _prep_in_maps(input, source_hids, W1, b1, W2)
    res = _run(in_maps)
    ctx = np.concatenate([res.results[c]["ctx_o"] for c in range(NCORES)], axis=0)
    attn = np.concatenate([res.results[c]["attn_o"] for c in range(NCORES)], axis=1)
    return ctx.astype(np.float32), attn.astype(np.float32)
